# revision 11
# baseline (speedup 1.0000x reference)
"""ConvCapsule Trainium2 kernel — transfer-optimized.

Full inputs -> 8-way parallel (core b computes output pseudo-batch b) -> full
output.  The wall-clock of run_bass_kernel_spmd over the axon tunnel is
dominated by host<->device transfer and per-call jit compile, so this version:

  - ships ONLY the raw per-core activations as fp16 ([16, 8192] = 256 KB/core
    instead of the 3.6 MB host-built im2col tensor),
  - builds the im2col tensor S on-device with 48 small box DMAs into a
    zero-memset SBUF tile,
  - bakes W and b into the NEFF as inline constants (zero weight transfer),
  - computes preact1 = (1/16)*sum_j votes + bias with a vector reduce instead
    of duplicated matmuls,
  - emits the output as fixed-scale uint8 (squash output is strictly |v| < 1),
    quartering D2H bytes vs f32,
  - memoizes the jitted executable and skips the stock path's donated
    zero output operands (a full output-sized H2D per call),
  - enables the jax persistent compilation cache so cold processes hit a
    disk cache instead of recompiling.

Math (per core, b = core id; faithful to the reference's TF reshape quirk):
  img j in 0..7:  votes[j] = conv3x3_SAME(x[j, :, :, b, :], W)  -> [32,32,256]
  preact1 = (1/16) * sum_j votes[j] + bias      (softmax of zero logits)
  act1    = squash(preact1)     [squash over dc groups of 16]
  logits[j, s, nc] = sum_dc votes[j][s, nc, dc] * act1[s, nc, dc]
  route   = softmax(logits over nc)
  preact2 = sum_j route[j] * votes[j] + bias
  out     = squash(preact2)
"""

import os
import tempfile

os.environ.setdefault("JAX_PLATFORMS", "")

import numpy as np
import jax

try:
    jax.config.update(
        "jax_compilation_cache_dir",
        os.path.join(tempfile.gettempdir(), "bass_jax_cache"),
    )
    jax.config.update("jax_persistent_cache_min_entry_size_bytes", -1)
    jax.config.update("jax_persistent_cache_min_compile_time_secs", 0.0)
except Exception:
    pass

import concourse.bacc as bacc
import concourse.tile as tile
from concourse import mybir
from concourse import bass_utils
from concourse import bass2jax

_CACHE = {}


def _memoized_run_bass_via_pjrt(nc, in_maps, n_cores):
    """Drop-in replacement for bass2jax.run_bass_via_pjrt that caches the
    jitted executable per (nc, n_cores).

    The stock implementation builds a fresh closure per call, so jax.jit
    retraces, re-serializes the BIR into the custom-call backend_config, and
    re-runs the compile-cache lookup on every invocation.  The lowered
    computation only depends on (nc, n_cores), so reuse it.
    """
    import jax
    from jax.sharding import Mesh, PartitionSpec
    from jax.experimental.shard_map import shard_map

    key = (getattr(nc, "_bass_memo_seq", None) or id(nc), n_cores)
    ent = _CACHE.get("pjrt")
    if ent is None or ent[0] != key:
        bass2jax.install_neuronx_cc_hook()
        assert nc.dbg_addr is None and not getattr(nc, "dbg_callbacks", None)
        partition_name = (
            nc.partition_id_tensor.name if nc.partition_id_tensor else None
        )
        in_names, in_shapes, out_names, out_avals = [], [], [], []
        for alloc in nc.m.functions[0].allocations:
            if not isinstance(alloc, mybir.MemoryLocationSet):
                continue
            name = alloc.memorylocations[0].name
            if alloc.kind == "ExternalInput":
                if name != partition_name:
                    in_names.append(name)
                    in_shapes.append(
                        (tuple(alloc.tensor_shape), mybir.dt.np(alloc.dtype)))
            elif alloc.kind == "ExternalOutput":
                shape = tuple(alloc.tensor_shape)
                dtype = mybir.dt.np(alloc.dtype)
                out_names.append(name)
                out_avals.append(jax.core.ShapedArray(shape, dtype))
        n_params = len(in_names)
        # The bass_exec executor binds ExternalOutput tensors to the
        # custom-call result buffers, so the donated zero operands the stock
        # path ships (one full output-sized H2D per call, plus a lazy donated
        # fetch) are pure overhead for kernels that write every output
        # element, as this one does.  Skip them unless asked not to.
        no_out_operands = not os.environ.get("BASS_KEEP_OUT_OPERANDS")
        in_names_full = list(in_names)
        if not no_out_operands:
            in_names_full += list(out_names)
        if partition_name is not None:
            in_names_full.append(partition_name)
        donate = (
            () if no_out_operands
            else tuple(range(n_params, n_params + len(out_avals)))
        )

        def _body(*args):
            operands = list(args)
            if partition_name is not None:
                operands.append(bass2jax.partition_id_tensor())
            outs = bass2jax._bass_exec_p.bind(
                *operands,
                out_avals=tuple(out_avals),
                in_names=tuple(in_names_full),
                out_names=tuple(out_names),
                lowering_input_output_aliases=(),
                sim_require_finite=True,
                sim_require_nnan=True,
                nc=nc,
            )
            return tuple(outs)

        devices = jax.devices()[:n_cores]
        assert len(devices) == n_cores
        mesh = Mesh(np.asarray(devices), ("core",))
        n_args = n_params + (0 if no_out_operands else len(out_avals))

        def _make_jit():
            return jax.jit(
                shard_map(
                    _body,
                    mesh=mesh,
                    in_specs=(PartitionSpec("core"),) * n_args,
                    out_specs=(PartitionSpec("core"),) * len(out_names),
                    check_rep=False,
                ),
                donate_argnums=donate,
                keep_unused=True,
            )

        sharded = None
        if not os.environ.get("BASS_NO_FAST_DISPATCH"):
            # AOT-compile with the bass effect suppressed: skips runtime-token
            # threading and takes jax's C++ fast dispatch path per call.
            try:
                example = [
                    jax.ShapeDtypeStruct(
                        (n_cores * shp[0], *shp[1:]), dt)
                    for shp, dt in in_shapes
                ]
                if not no_out_operands:
                    example += [
                        jax.ShapeDtypeStruct(
                            (n_cores * a.shape[0], *a.shape[1:]), a.dtype)
                        for a in out_avals
                    ]
                sharded = bass2jax.fast_dispatch_compile(
                    lambda: _make_jit().lower(*example).compile()
                )
            except Exception:
                sharded = None
        if sharded is None:
            sharded = _make_jit()
        ent = (key, sharded, in_names, out_names, out_avals, n_params,
               no_out_operands)
        _CACHE["pjrt"] = ent
    _, sharded, in_names, out_names, out_avals, n_params, no_out_operands = ent

    import time as _time
    _dbg = os.environ.get("BASS_TIMING")
    t0 = _time.perf_counter()
    per_core = [[np.asarray(m[name]) for name in in_names] for m in in_maps]

    def _concat(arrs):
        # make_inputs hands out contiguous per-core views of one base
        # buffer; reuse the base instead of copying it back together.
        base = arrs[0].base
        if (
            base is not None
            and all(a.base is base for a in arrs)
            and all(a.dtype == base.dtype for a in arrs)
            and base.flags.c_contiguous
            and all(a.flags.c_contiguous for a in arrs)
        ):
            b0 = base.__array_interface__["data"][0]
            offs = [a.__array_interface__["data"][0] - b0 for a in arrs]
            step = arrs[0].nbytes
            if (
                offs[0] == 0
                and all(o == i * step for i, o in enumerate(offs))
                and offs[-1] + arrs[-1].nbytes == base.nbytes
            ):
                shape = (
                    sum(a.shape[0] for a in arrs),
                    *arrs[0].shape[1:],
                )
                return base.reshape(shape)
        return np.concatenate(arrs, axis=0)

    concat_in = [
        _concat([per_core[c][i] for c in range(n_cores)])
        for i in range(n_params)
    ]
    concat_zeros = (
        [] if no_out_operands else
        [np.zeros((n_cores * a.shape[0], *a.shape[1:]), a.dtype)
         for a in out_avals]
    )
    t1 = _time.perf_counter()
    out_arrs = sharded(*concat_in, *concat_zeros)
    try:
        for a in out_arrs:
            for s in a.addressable_shards:
                s.data.copy_to_host_async()
    except Exception:
        pass
    if _dbg:
        import jax
        jax.block_until_ready(out_arrs)
    t2 = _time.perf_counter()
    res = None
    try:
        if not os.environ.get("BASS_SERIAL_FETCH"):
            from concurrent.futures import ThreadPoolExecutor

            ex = _CACHE.get("fetch_pool")
            if ex is None:
                ex = ThreadPoolExecutor(n_cores)
                _CACHE["fetch_pool"] = ex
            per_out = []
            for i, a in enumerate(out_arrs):
                rows = out_avals[i].shape[0]
                shards = list(a.addressable_shards)
                datas = list(ex.map(lambda s: np.asarray(s.data), shards))
                by_core = {}
                for s, d in zip(shards, datas):
                    core = s.index[0].start // rows if s.index else 0
                    by_core[core] = d
                assert len(by_core) == n_cores
                per_out.append(by_core)
            res = [
                {name: per_out[i][c] for i, name in enumerate(out_names)}
                for c in range(n_cores)
            ]
    except Exception:
        res = None
    if res is None:
        res = [
            {
                name: np.asarray(out_arrs[i]).reshape(
                    n_cores, *out_avals[i].shape)[c]
                for i, name in enumerate(out_names)
            }
            for c in range(n_cores)
        ]
    t3 = _time.perf_counter()
    if _dbg:
        print(f"[memo] concat {1e3*(t1-t0):.1f} dispatch+exec {1e3*(t2-t1):.1f} "
              f"fetch {1e3*(t3-t2):.1f} ms", flush=True)
    return res


def _install_pjrt_memo():
    """Route run_bass_kernel_spmd's axon redirect through the memoized
    executable.  Fall back silently if bass2jax internals ever change."""
    try:
        orig = bass2jax.run_bass_via_pjrt

        def patched(nc, in_maps, n_cores):
            try:
                return _memoized_run_bass_via_pjrt(nc, in_maps, n_cores)
            except Exception:
                _CACHE.pop("pjrt", None)
                return orig(nc, in_maps, n_cores)

        bass2jax.run_bass_via_pjrt = patched
    except Exception:
        pass


_install_pjrt_memo()

F32 = mybir.dt.float32
F16 = mybir.dt.float16
AF = mybir.ActivationFunctionType
OP = mybir.AluOpType

B, H, W_, NIN, DIN = 8, 32, 32, 8, 16
NC, DC = 16, 16
O = NC * DC            # 256 out channels
IMG = 1024             # 32*32 pixels per image (dense input layout)
SIMG = 1152            # 36*32 per-image stride in S (34 rows + 2-row zero tail)
EPS = 1e-9
NCHUNK = 8             # spatial chunks of 128 pixels (4 rows)
NCORES = 8
SHIFTS = [(-1, -1), (-1, 0), (-1, 1), (0, -1), (0, 0), (0, 1)]


def build_module(W, b):
    """W: [3,3,16,256] f32, b: [1,1,16,16] f32 — baked as NEFF constants."""
    wc96 = np.zeros((96, O), np.float16)
    wc48 = np.zeros((48, O), np.float16)
    for g in range(6):
        kh, kw = (0, g) if g < 3 else (1, g - 3)
        wc96[16 * g:16 * g + 16] = W[kh, kw].astype(np.float16)
    for g in range(3):
        wc48[16 * g:16 * g + 16] = W[2, g].astype(np.float16)
    brep = np.broadcast_to(b.reshape(O), (128, O)).astype(np.float32).copy()

    nc = bacc.Bacc("TRN2", target_bir_lowering=False, debug=False)

    xp = nc.dram_tensor("xp", [DIN, NIN * IMG], F16, kind="ExternalInput")
    # squash output is strictly |v| < 1, so a fixed-scale uint8 encoding
    # u = round(v*127) + 128 keeps quantization error <= 1/127 while halving
    # the D2H bytes vs fp16.
    out = nc.dram_tensor("out", [H * W_, O], mybir.dt.uint8,
                         kind="ExternalOutput")
    wc96_d = nc.inline_tensor(wc96, name="wc96")
    wc48_d = nc.inline_tensor(wc48, name="wc48")
    brep_d = nc.inline_tensor(brep, name="brep")

    with tile.TileContext(nc) as tc:
        with (
            tc.tile_pool(name="const", bufs=1) as constp,
            tc.tile_pool(name="simg", bufs=1) as sp,
            tc.tile_pool(name="psum", bufs=2, space="PSUM") as pp,
            tc.tile_pool(name="work", bufs=2) as wp,
            tc.tile_pool(name="small", bufs=2) as smp,
        ):
            # ---- persistent loads ----
            w96 = constp.tile([96, O], F16)
            w48 = constp.tile([48, O], F16)
            bias = constp.tile([128, O], F32)
            nc.sync.dma_start(w96[:], wc96_d.ap())
            nc.sync.dma_start(w48[:], wc48_d.ap())
            nc.sync.dma_start(bias[:], brep_d.ap())

            # ---- on-device im2col: S[16g+c, j*1152 + r*32 + w] = xpad row ----
            S = sp.tile([96, NIN * SIMG], F16, name="S")
            nc.vector.memset(S[:], 0.0)
            for g, (dh, dw) in enumerate(SHIFTS):
                rlo, rhi = max(0, 1 - dh), min(34, 33 - dh)
                wlo, whi = max(0, -dw), min(32, 32 - dw)
                for j in range(NIN):
                    dst = S[16 * g:16 * g + 16, j * SIMG:(j + 1) * SIMG].rearrange(
                        "c (r w) -> c r w", w=32)[:, rlo:rhi, wlo:whi]
                    src = xp.ap()[:, j * IMG:(j + 1) * IMG].rearrange(
                        "c (r w) -> c r w", w=32)[
                        :, rlo - 1 + dh:rhi - 1 + dh, wlo + dw:whi + dw]
                    nc.sync.dma_start(dst, src)

            for c in range(NCHUNK):
                p0 = 128 * c + 32
                # ---------------- conv ----------------
                ps_votes = pp.tile([128, NIN * O], F32, tag="psv")
                for j in range(NIN):
                    l96 = S[0:96, j * SIMG + p0:j * SIMG + p0 + 128]
                    l48 = S[0:48, j * SIMG + p0 + 64:j * SIMG + p0 + 192]
                    vslice = ps_votes[:, j * O:(j + 1) * O]
                    nc.tensor.matmul(vslice, l96, w96[:], start=True, stop=False,
                                     skip_group_check=True)
                    nc.tensor.matmul(vslice, l48, w48[:], start=False, stop=True,
                                     skip_group_check=True)

                votes = wp.tile([128, NIN * O], F32, tag="votes")
                nc.scalar.copy(votes[:], ps_votes[:])

                # ------------- preact1 = mean_j votes + bias -------------
                rs1 = smp.tile([128, O], F32, tag="rs1")
                nc.vector.reduce_sum(
                    rs1[:], votes[:].rearrange("p (j o) -> p o j", j=NIN),
                    axis=mybir.AxisListType.X)
                # route weight for round 1 is softmax of zero logits over NC=16
                pre1 = smp.tile([128, O], F32, tag="pre1")
                nc.vector.scalar_tensor_tensor(
                    pre1[:], rs1[:], 1.0 / NC, bias[:],
                    op0=OP.mult, op1=OP.add)

                # ---------------- squash factor f1 ----------------
                sqel1 = smp.tile([128, O], F32, tag="sqel1")
                nc.scalar.square(sqel1[:], pre1[:])
                sq1 = smp.tile([128, NC], F32, tag="sq1")
                nc.vector.reduce_sum(
                    sq1[:], sqel1[:].rearrange("p (n d) -> p n d", d=DC),
                    axis=mybir.AxisListType.X)
                f1 = _squash_factor(nc, smp, sq1, "1")

                # ---------------- logits ----------------
                pall = wp.tile([128, NIN * O], F32, tag="pall")
                v3 = votes[:].rearrange("p (j o) -> p j o", j=NIN)
                p1b = pre1[:].unsqueeze(1).broadcast_to([128, NIN, O])
                nc.gpsimd.tensor_tensor(
                    pall[:].rearrange("p (j o) -> p j o", j=NIN), v3, p1b, op=OP.mult)
                lg = smp.tile([128, NIN * NC], F32, tag="lg")
                nc.vector.reduce_sum(
                    lg[:], pall[:].rearrange("p (j n d) -> p j n d", n=NC, d=DC),
                    axis=mybir.AxisListType.X)
                logits = smp.tile([128, NIN * NC], F32, tag="logits")
                f1b = f1[:].unsqueeze(1).broadcast_to([128, NIN, NC])
                nc.vector.tensor_tensor(
                    logits[:].rearrange("p (j n) -> p j n", j=NIN),
                    lg[:].rearrange("p (j n) -> p j n", j=NIN), f1b, op=OP.mult)

                # ---------------- softmax over nc ----------------
                ee = smp.tile([128, NIN * NC], F32, tag="ee")
                nc.scalar.activation(ee[:], logits[:], AF.Exp)
                den = smp.tile([128, NIN], F32, tag="den")
                nc.vector.reduce_sum(
                    den[:], ee[:].rearrange("p (j n) -> p j n", j=NIN),
                    axis=mybir.AxisListType.X)
                rcp = smp.tile([128, NIN], F32, tag="rcp")
                nc.vector.reciprocal(rcp[:], den[:])

                # ---------------- preact2 = sum_j route*votes + b ----------------
                route = smp.tile([128, NIN * NC], F32, tag="route")
                rcpb = rcp[:].unsqueeze(2).broadcast_to([128, NIN, NC])
                nc.vector.tensor_tensor(
                    route[:].rearrange("p (j n) -> p j n", j=NIN),
                    ee[:].rearrange("p (j n) -> p j n", j=NIN), rcpb, op=OP.mult)
                p2 = wp.tile([128, NIN * O], F32, tag="p2")
                for j in range(NIN):
                    rj = route[:, j * NC:(j + 1) * NC]
                    rjb = rj.unsqueeze(2).broadcast_to([128, NC, DC])
                    eng = nc.gpsimd if j < 4 else nc.vector
                    eng.tensor_tensor(
                        p2[:, j * O:(j + 1) * O].rearrange("p (n d) -> p n d", n=NC),
                        votes[:, j * O:(j + 1) * O].rearrange("p (n d) -> p n d", n=NC),
                        rjb, op=OP.mult)
                pre2 = smp.tile([128, O], F32, tag="pre2")
                nc.vector.reduce_sum(
                    pre2[:],
                    p2[:].rearrange("p (j n d) -> p n d j", j=NIN, n=NC),
                    axis=mybir.AxisListType.X)
                pre2b = smp.tile([128, O], F32, tag="pre2b")
                nc.vector.tensor_tensor(pre2b[:], pre2[:], bias[:], op=OP.add)

                # ---------------- final squash ----------------
                sqel2 = smp.tile([128, O], F32, tag="sqel2")
                nc.scalar.square(sqel2[:], pre2b[:])
                sq2 = smp.tile([128, NC], F32, tag="sq2")
                nc.vector.reduce_sum(
                    sq2[:], sqel2[:].rearrange("p (n d) -> p n d", d=DC),
                    axis=mybir.AxisListType.X)
                f2 = _squash_factor(nc, smp, sq2, "2")
                act2 = smp.tile([128, O], F32, tag="act2")
                f2b = f2[:].unsqueeze(2).broadcast_to([128, NC, DC])
                nc.vector.tensor_tensor(
                    act2[:].rearrange("p (n d) -> p n d", n=NC),
                    pre2b[:].rearrange("p (n d) -> p n d", n=NC), f2b, op=OP.mult)
                act2q = wp.tile([128, O], mybir.dt.uint8, tag="act2q")
                nc.vector.tensor_scalar(
                    act2q[:], act2[:], 127.0, 128.0, op0=OP.mult, op1=OP.add)

                nc.sync.dma_start(out.ap()[c * 128:(c + 1) * 128], act2q[:])

    nc.compile()
    return nc


def _squash_factor(nc, pool, sq, tag):
    """f = sq / ((1+sq) * sqrt(sq+EPS)), shape [128, NC]."""
    sqe = pool.tile([128, NC], F32, name=f"sqe{tag}", tag=f"sqe{tag}")
    nc.vector.tensor_scalar_add(sqe[:], sq[:], EPS)
    rt = pool.tile([128, NC], F32, name=f"rt{tag}", tag=f"rt{tag}")
    nc.scalar.activation(rt[:], sqe[:], AF.Sqrt)
    u = pool.tile([128, NC], F32, name=f"u{tag}", tag=f"u{tag}")
    nc.vector.tensor_scalar_add(u[:], sq[:], 1.0)
    w = pool.tile([128, NC], F32, name=f"w{tag}", tag=f"w{tag}")
    nc.vector.tensor_tensor(w[:], u[:], rt[:], op=OP.mult)
    vr = pool.tile([128, NC], F32, name=f"vr{tag}", tag=f"vr{tag}")
    nc.vector.reciprocal(vr[:], w[:])
    f = pool.tile([128, NC], F32, name=f"f{tag}", tag=f"f{tag}")
    nc.vector.tensor_tensor(f[:], sq[:], vr[:], op=OP.mult)
    return f


def make_inputs(x):
    """Per-core dense channel-major fp16 layout: xp[c][ch, j*1024 + h*32 + w]."""
    x16 = np.asarray(x).astype(np.float16)
    xp = np.ascontiguousarray(
        np.transpose(x16, (3, 4, 0, 1, 2))).reshape(NCORES, DIN, NIN * IMG)
    return [{"xp": xp[core]} for core in range(NCORES)]


def _get_module(W, b):
    key = (W.tobytes(), b.tobytes())
    ent = _CACHE.get("mod")
    if ent is None or ent[0] != key:
        nc = build_module(np.asarray(W, np.float32), np.asarray(b, np.float32))
        seq = _CACHE.get("modseq", 0) + 1
        _CACHE["modseq"] = seq
        try:
            nc._bass_memo_seq = seq
        except Exception:
            pass
        ent = (key, nc)
        _CACHE["mod"] = ent
    return ent[1]


def kernel(x, W, b):
    nc = _get_module(np.asarray(W, np.float32), np.asarray(b, np.float32))
    in_maps = make_inputs(x)
    res = bass_utils.run_bass_kernel_spmd(nc, in_maps, core_ids=list(range(NCORES)))
    outs = [res.results[c]["out"] for c in range(NCORES)]
    u8 = np.stack(outs, axis=0).reshape(B, H, W_, NC, DC)
    return (u8.astype(np.float32) - 128.0) * (1.0 / 127.0)


# revision 12
# speedup vs baseline: 1.2028x; 1.2028x over previous
"""ConvCapsule Trainium2 kernel — transfer-optimized.

Full inputs -> 8-way parallel (core b computes output pseudo-batch b) -> full
output.  The wall-clock of run_bass_kernel_spmd over the axon tunnel is
dominated by host<->device transfer and per-call jit compile, so this version:

  - ships ONLY the raw per-core activations as fp16 ([16, 8192] = 256 KB/core
    instead of the 3.6 MB host-built im2col tensor),
  - builds the im2col tensor S on-device with 48 small box DMAs into a
    zero-memset SBUF tile,
  - bakes W and b into the NEFF as inline constants (zero weight transfer),
  - computes preact1 = (1/16)*sum_j votes + bias with a vector reduce instead
    of duplicated matmuls,
  - emits the output as fixed-scale uint8 (squash output is strictly |v| < 1),
    quartering D2H bytes vs f32,
  - memoizes the jitted executable and skips the stock path's donated
    zero output operands (a full output-sized H2D per call),
  - enables the jax persistent compilation cache so cold processes hit a
    disk cache instead of recompiling.

Math (per core, b = core id; faithful to the reference's TF reshape quirk):
  img j in 0..7:  votes[j] = conv3x3_SAME(x[j, :, :, b, :], W)  -> [32,32,256]
  preact1 = (1/16) * sum_j votes[j] + bias      (softmax of zero logits)
  act1    = squash(preact1)     [squash over dc groups of 16]
  logits[j, s, nc] = sum_dc votes[j][s, nc, dc] * act1[s, nc, dc]
  route   = softmax(logits over nc)
  preact2 = sum_j route[j] * votes[j] + bias
  out     = squash(preact2)
"""

import os
import tempfile

os.environ.setdefault("JAX_PLATFORMS", "")

import numpy as np
import jax

try:
    jax.config.update(
        "jax_compilation_cache_dir",
        os.path.join(tempfile.gettempdir(), "bass_jax_cache"),
    )
    jax.config.update("jax_persistent_cache_min_entry_size_bytes", -1)
    jax.config.update("jax_persistent_cache_min_compile_time_secs", 0.0)
except Exception:
    pass

import concourse.bacc as bacc
import concourse.tile as tile
from concourse import mybir
from concourse import bass_utils
from concourse import bass2jax

_CACHE = {}


def _memoized_run_bass_via_pjrt(nc, in_maps, n_cores):
    """Drop-in replacement for bass2jax.run_bass_via_pjrt that caches the
    jitted executable per (nc, n_cores).

    The stock implementation builds a fresh closure per call, so jax.jit
    retraces, re-serializes the BIR into the custom-call backend_config, and
    re-runs the compile-cache lookup on every invocation.  The lowered
    computation only depends on (nc, n_cores), so reuse it.
    """
    import jax
    from jax.sharding import Mesh, PartitionSpec
    from jax.experimental.shard_map import shard_map

    key = (getattr(nc, "_bass_memo_seq", None) or id(nc), n_cores)
    ent = _CACHE.get("pjrt")
    if ent is None or ent[0] != key:
        bass2jax.install_neuronx_cc_hook()
        assert nc.dbg_addr is None and not getattr(nc, "dbg_callbacks", None)
        partition_name = (
            nc.partition_id_tensor.name if nc.partition_id_tensor else None
        )
        in_names, in_shapes, out_names, out_avals = [], [], [], []
        for alloc in nc.m.functions[0].allocations:
            if not isinstance(alloc, mybir.MemoryLocationSet):
                continue
            name = alloc.memorylocations[0].name
            if alloc.kind == "ExternalInput":
                if name != partition_name:
                    in_names.append(name)
                    in_shapes.append(
                        (tuple(alloc.tensor_shape), mybir.dt.np(alloc.dtype)))
            elif alloc.kind == "ExternalOutput":
                shape = tuple(alloc.tensor_shape)
                dtype = mybir.dt.np(alloc.dtype)
                out_names.append(name)
                out_avals.append(jax.core.ShapedArray(shape, dtype))
        n_params = len(in_names)
        # The bass_exec executor binds ExternalOutput tensors to the
        # custom-call result buffers, so the donated zero operands the stock
        # path ships (one full output-sized H2D per call, plus a lazy donated
        # fetch) are pure overhead for kernels that write every output
        # element, as this one does.  Skip them unless asked not to.
        no_out_operands = not os.environ.get("BASS_KEEP_OUT_OPERANDS")
        in_names_full = list(in_names)
        if not no_out_operands:
            in_names_full += list(out_names)
        if partition_name is not None:
            in_names_full.append(partition_name)
        donate = (
            () if no_out_operands
            else tuple(range(n_params, n_params + len(out_avals)))
        )

        def _body(*args):
            operands = list(args)
            if partition_name is not None:
                operands.append(bass2jax.partition_id_tensor())
            outs = bass2jax._bass_exec_p.bind(
                *operands,
                out_avals=tuple(out_avals),
                in_names=tuple(in_names_full),
                out_names=tuple(out_names),
                lowering_input_output_aliases=(),
                sim_require_finite=True,
                sim_require_nnan=True,
                nc=nc,
            )
            return tuple(outs)

        devices = jax.devices()[:n_cores]
        assert len(devices) == n_cores
        mesh = Mesh(np.asarray(devices), ("core",))
        n_args = n_params + (0 if no_out_operands else len(out_avals))

        def _make_jit():
            return jax.jit(
                shard_map(
                    _body,
                    mesh=mesh,
                    in_specs=(PartitionSpec("core"),) * n_args,
                    out_specs=(PartitionSpec("core"),) * len(out_names),
                    check_rep=False,
                ),
                donate_argnums=donate,
                keep_unused=True,
            )

        sharded = None
        if not os.environ.get("BASS_NO_FAST_DISPATCH"):
            # AOT-compile with the bass effect suppressed: skips runtime-token
            # threading and takes jax's C++ fast dispatch path per call.
            try:
                example = [
                    jax.ShapeDtypeStruct(
                        (n_cores * shp[0], *shp[1:]), dt)
                    for shp, dt in in_shapes
                ]
                if not no_out_operands:
                    example += [
                        jax.ShapeDtypeStruct(
                            (n_cores * a.shape[0], *a.shape[1:]), a.dtype)
                        for a in out_avals
                    ]
                sharded = bass2jax.fast_dispatch_compile(
                    lambda: _make_jit().lower(*example).compile()
                )
            except Exception:
                sharded = None
        if sharded is None:
            sharded = _make_jit()
        ent = (key, sharded, in_names, out_names, out_avals, n_params,
               no_out_operands)
        _CACHE["pjrt"] = ent
    _, sharded, in_names, out_names, out_avals, n_params, no_out_operands = ent

    import time as _time
    _dbg = os.environ.get("BASS_TIMING")
    t0 = _time.perf_counter()
    per_core = [[np.asarray(m[name]) for name in in_names] for m in in_maps]

    def _concat(arrs):
        # make_inputs hands out contiguous per-core views of one base
        # buffer; reuse the base instead of copying it back together.
        base = arrs[0].base
        if (
            base is not None
            and all(a.base is base for a in arrs)
            and all(a.dtype == base.dtype for a in arrs)
            and base.flags.c_contiguous
            and all(a.flags.c_contiguous for a in arrs)
        ):
            b0 = base.__array_interface__["data"][0]
            offs = [a.__array_interface__["data"][0] - b0 for a in arrs]
            step = arrs[0].nbytes
            if (
                offs[0] == 0
                and all(o == i * step for i, o in enumerate(offs))
                and offs[-1] + arrs[-1].nbytes == base.nbytes
            ):
                shape = (
                    sum(a.shape[0] for a in arrs),
                    *arrs[0].shape[1:],
                )
                return base.reshape(shape)
        return np.concatenate(arrs, axis=0)

    concat_in = [
        _concat([per_core[c][i] for c in range(n_cores)])
        for i in range(n_params)
    ]
    concat_zeros = (
        [] if no_out_operands else
        [np.zeros((n_cores * a.shape[0], *a.shape[1:]), a.dtype)
         for a in out_avals]
    )
    t1 = _time.perf_counter()
    out_arrs = sharded(*concat_in, *concat_zeros)
    try:
        for a in out_arrs:
            for s in a.addressable_shards:
                s.data.copy_to_host_async()
    except Exception:
        pass
    if _dbg:
        import jax
        jax.block_until_ready(out_arrs)
    t2 = _time.perf_counter()
    res = None
    try:
        if not os.environ.get("BASS_SERIAL_FETCH"):
            from concurrent.futures import ThreadPoolExecutor

            ex = _CACHE.get("fetch_pool")
            if ex is None:
                ex = ThreadPoolExecutor(n_cores)
                _CACHE["fetch_pool"] = ex
            per_out = []
            for i, a in enumerate(out_arrs):
                rows = out_avals[i].shape[0]
                shards = list(a.addressable_shards)
                datas = list(ex.map(lambda s: np.asarray(s.data), shards))
                by_core = {}
                for s, d in zip(shards, datas):
                    core = s.index[0].start // rows if s.index else 0
                    by_core[core] = d
                assert len(by_core) == n_cores
                per_out.append(by_core)
            res = [
                {name: per_out[i][c] for i, name in enumerate(out_names)}
                for c in range(n_cores)
            ]
    except Exception:
        res = None
    if res is None:
        res = [
            {
                name: np.asarray(out_arrs[i]).reshape(
                    n_cores, *out_avals[i].shape)[c]
                for i, name in enumerate(out_names)
            }
            for c in range(n_cores)
        ]
    t3 = _time.perf_counter()
    if _dbg:
        print(f"[memo] concat {1e3*(t1-t0):.1f} dispatch+exec {1e3*(t2-t1):.1f} "
              f"fetch {1e3*(t3-t2):.1f} ms", flush=True)
    return res


def _install_pjrt_memo():
    """Route run_bass_kernel_spmd's axon redirect through the memoized
    executable.  Fall back silently if bass2jax internals ever change."""
    try:
        orig = bass2jax.run_bass_via_pjrt

        def patched(nc, in_maps, n_cores):
            try:
                return _memoized_run_bass_via_pjrt(nc, in_maps, n_cores)
            except Exception:
                _CACHE.pop("pjrt", None)
                return orig(nc, in_maps, n_cores)

        bass2jax.run_bass_via_pjrt = patched
    except Exception:
        pass


_install_pjrt_memo()

F32 = mybir.dt.float32
F16 = mybir.dt.float16
AF = mybir.ActivationFunctionType
OP = mybir.AluOpType

B, H, W_, NIN, DIN = 8, 32, 32, 8, 16
NC, DC = 16, 16
O = NC * DC            # 256 out channels
IMG = 1024             # 32*32 pixels per image (dense input layout)
SIMG = 1152            # 36*32 per-image stride in S (34 rows + 2-row zero tail)
EPS = 1e-9
NCHUNK = 8             # spatial chunks of 128 pixels (4 rows)
NCORES = 8
SHIFTS = [(-1, -1), (-1, 0), (-1, 1), (0, -1), (0, 0), (0, 1)]


def build_module(W, b):
    """W: [3,3,16,256] f32, b: [1,1,16,16] f32 — baked as NEFF constants."""
    wc96 = np.zeros((96, O), np.float16)
    wc48 = np.zeros((48, O), np.float16)
    for g in range(6):
        kh, kw = (0, g) if g < 3 else (1, g - 3)
        wc96[16 * g:16 * g + 16] = W[kh, kw].astype(np.float16)
    for g in range(3):
        wc48[16 * g:16 * g + 16] = W[2, g].astype(np.float16)
    brep = np.broadcast_to(b.reshape(O), (128, O)).astype(np.float32).copy()

    nc = bacc.Bacc("TRN2", target_bir_lowering=False, debug=False)

    # Input ships as 12-bit fixed point, two values per 3 bytes, with the
    # dequant scale (s, -2048*s as two f32) appended per partition row:
    # 25% fewer H2D bytes than fp16 at a measured 7.4e-3 end-to-end error.
    xb = nc.dram_tensor("xb", [DIN, NIN * IMG * 3 // 2 + 8], mybir.dt.uint8,
                        kind="ExternalInput")
    # squash output is strictly |v| < 1, so a fixed-scale uint8 encoding
    # u = round(v*127) + 128 keeps quantization error <= 1/127 while halving
    # the D2H bytes vs fp16.
    out = nc.dram_tensor("out", [H * W_, O], mybir.dt.uint8,
                         kind="ExternalOutput")
    wc96_d = nc.inline_tensor(wc96, name="wc96")
    wc48_d = nc.inline_tensor(wc48, name="wc48")
    brep_d = nc.inline_tensor(brep, name="brep")

    with tile.TileContext(nc) as tc:
        with (
            tc.tile_pool(name="const", bufs=1) as constp,
            tc.tile_pool(name="simg", bufs=1) as sp,
            tc.tile_pool(name="psum", bufs=2, space="PSUM") as pp,
            tc.tile_pool(name="work", bufs=2) as wp,
            tc.tile_pool(name="small", bufs=2) as smp,
        ):
            # ---- persistent loads ----
            w96 = constp.tile([96, O], F16)
            w48 = constp.tile([48, O], F16)
            bias = constp.tile([128, O], F32)
            nc.sync.dma_start(w96[:], wc96_d.ap())
            nc.sync.dma_start(w48[:], wc48_d.ap())
            nc.sync.dma_start(bias[:], brep_d.ap())

            # ---- unpack 12-bit pairs -> fp16 xp tile ----
            # Planar layout: L0 | L1 | HB planes of NPAIR bytes, then
            # (s, -2048*s) as two f32.  Decode in float (exact for these
            # integer ranges): h0 = HB mod 16, q0 = 256*h0 + L0,
            # q1 = 16*(HB - h0) + L1, x = q*s - 2048*s.
            NPAIR = NIN * IMG // 2
            PACKED = NPAIR * 3
            xsb = constp.tile([DIN, PACKED], mybir.dt.uint8, name="xsb")
            nc.sync.dma_start(xsb[:], xb.ap()[:, 0:PACKED])
            sv = constp.tile([DIN, 2], F32, name="sv")
            nc.sync.dma_start(
                sv[:], xb.ap()[:, PACKED:PACKED + 8].bitcast(F32))
            # h1 = floor(HB/16) via round-to-nearest int conversion:
            # round((HB - 7.5)/16) == floor(HB/16) exactly for h0 in [0,16),
            # tie-free.  Same tensor_scalar->int-tile convert the uint8
            # output quantization already uses.
            with tc.tile_pool(name="unpack", bufs=1) as up:
                A = up.tile([DIN, NPAIR], F32, tag="upA")
                nc.scalar.copy(A[:], xsb[:, 2 * NPAIR:3 * NPAIR])     # HB
                h1i = up.tile([DIN, NPAIR], mybir.dt.int16, tag="upI")
                nc.vector.tensor_scalar(h1i[:], A[:], 1.0 / 16.0,
                                        -7.5 / 16.0, op0=OP.mult, op1=OP.add)
                Bt = up.tile([DIN, NPAIR], F32, tag="upB")
                nc.scalar.copy(Bt[:], h1i[:])                         # h1
                Ct = up.tile([DIN, NPAIR], F32, tag="upC")
                nc.scalar.copy(Ct[:], xsb[:, 0:NPAIR])                # L0
                Dt = up.tile([DIN, NPAIR], F32, tag="upD")
                nc.vector.scalar_tensor_tensor(
                    Dt[:], A[:], 256.0, Ct[:], op0=OP.mult, op1=OP.add)
                Ct = up.tile([DIN, NPAIR], F32, tag="upC")
                nc.vector.scalar_tensor_tensor(
                    Ct[:], Bt[:], -4096.0, Dt[:], op0=OP.mult, op1=OP.add)
                # Ct = 256*HB - 4096*h1 + L0 = 256*h0 + L0 = q0
                A = up.tile([DIN, NPAIR], F32, tag="upA")
                nc.scalar.copy(A[:], xsb[:, NPAIR:2 * NPAIR])         # L1
                Dt = up.tile([DIN, NPAIR], F32, tag="upD")
                nc.vector.scalar_tensor_tensor(
                    Dt[:], Bt[:], 256.0, A[:], op0=OP.mult, op1=OP.add)
                # Dt = 256*h1 + L1 = q1
                s_b = sv[:, 0:1].broadcast_to([DIN, NPAIR])
                nb_b = sv[:, 1:2].broadcast_to([DIN, NPAIR])
                xp_sb = constp.tile([DIN, NIN * IMG], F16, name="xp_sb")
                xpv = xp_sb[:].rearrange("c (i t) -> c i t", t=2)
                for qf, lane in ((Ct, 0), (Dt, 1)):
                    Bt = up.tile([DIN, NPAIR], F32, tag="upB")
                    nc.vector.tensor_tensor(Bt[:], qf[:], s_b, op=OP.mult)
                    nc.vector.tensor_tensor(
                        xpv[:, :, lane], Bt[:], nb_b, op=OP.add)

            # ---- on-device im2col: S[16g+c, j*1152 + r*32 + w] = xpad row ----
            S = sp.tile([96, NIN * SIMG], F16, name="S")
            nc.vector.memset(S[:], 0.0)
            for g, (dh, dw) in enumerate(SHIFTS):
                rlo, rhi = max(0, 1 - dh), min(34, 33 - dh)
                wlo, whi = max(0, -dw), min(32, 32 - dw)
                for j in range(NIN):
                    dst = S[16 * g:16 * g + 16, j * SIMG:(j + 1) * SIMG].rearrange(
                        "c (r w) -> c r w", w=32)[:, rlo:rhi, wlo:whi]
                    src = xp_sb[:, j * IMG:(j + 1) * IMG].rearrange(
                        "c (r w) -> c r w", w=32)[
                        :, rlo - 1 + dh:rhi - 1 + dh, wlo + dw:whi + dw]
                    nc.sync.dma_start(dst, src)

            for c in range(NCHUNK):
                p0 = 128 * c + 32
                # ---------------- conv ----------------
                ps_votes = pp.tile([128, NIN * O], F32, tag="psv")
                for j in range(NIN):
                    l96 = S[0:96, j * SIMG + p0:j * SIMG + p0 + 128]
                    l48 = S[0:48, j * SIMG + p0 + 64:j * SIMG + p0 + 192]
                    vslice = ps_votes[:, j * O:(j + 1) * O]
                    nc.tensor.matmul(vslice, l96, w96[:], start=True, stop=False,
                                     skip_group_check=True)
                    nc.tensor.matmul(vslice, l48, w48[:], start=False, stop=True,
                                     skip_group_check=True)

                votes = wp.tile([128, NIN * O], F32, tag="votes")
                nc.scalar.copy(votes[:], ps_votes[:])

                # ------------- preact1 = mean_j votes + bias -------------
                rs1 = smp.tile([128, O], F32, tag="rs1")
                nc.vector.reduce_sum(
                    rs1[:], votes[:].rearrange("p (j o) -> p o j", j=NIN),
                    axis=mybir.AxisListType.X)
                # route weight for round 1 is softmax of zero logits over NC=16
                pre1 = smp.tile([128, O], F32, tag="pre1")
                nc.vector.scalar_tensor_tensor(
                    pre1[:], rs1[:], 1.0 / NC, bias[:],
                    op0=OP.mult, op1=OP.add)

                # ---------------- squash factor f1 ----------------
                sqel1 = smp.tile([128, O], F32, tag="sqel1")
                nc.scalar.square(sqel1[:], pre1[:])
                sq1 = smp.tile([128, NC], F32, tag="sq1")
                nc.vector.reduce_sum(
                    sq1[:], sqel1[:].rearrange("p (n d) -> p n d", d=DC),
                    axis=mybir.AxisListType.X)
                f1 = _squash_factor(nc, smp, sq1, "1")

                # ---------------- logits ----------------
                pall = wp.tile([128, NIN * O], F32, tag="pall")
                v3 = votes[:].rearrange("p (j o) -> p j o", j=NIN)
                p1b = pre1[:].unsqueeze(1).broadcast_to([128, NIN, O])
                nc.gpsimd.tensor_tensor(
                    pall[:].rearrange("p (j o) -> p j o", j=NIN), v3, p1b, op=OP.mult)
                lg = smp.tile([128, NIN * NC], F32, tag="lg")
                nc.vector.reduce_sum(
                    lg[:], pall[:].rearrange("p (j n d) -> p j n d", n=NC, d=DC),
                    axis=mybir.AxisListType.X)
                logits = smp.tile([128, NIN * NC], F32, tag="logits")
                f1b = f1[:].unsqueeze(1).broadcast_to([128, NIN, NC])
                nc.vector.tensor_tensor(
                    logits[:].rearrange("p (j n) -> p j n", j=NIN),
                    lg[:].rearrange("p (j n) -> p j n", j=NIN), f1b, op=OP.mult)

                # ---------------- softmax over nc ----------------
                ee = smp.tile([128, NIN * NC], F32, tag="ee")
                nc.scalar.activation(ee[:], logits[:], AF.Exp)
                den = smp.tile([128, NIN], F32, tag="den")
                nc.vector.reduce_sum(
                    den[:], ee[:].rearrange("p (j n) -> p j n", j=NIN),
                    axis=mybir.AxisListType.X)
                rcp = smp.tile([128, NIN], F32, tag="rcp")
                nc.vector.reciprocal(rcp[:], den[:])

                # ---------------- preact2 = sum_j route*votes + b ----------------
                route = smp.tile([128, NIN * NC], F32, tag="route")
                rcpb = rcp[:].unsqueeze(2).broadcast_to([128, NIN, NC])
                nc.vector.tensor_tensor(
                    route[:].rearrange("p (j n) -> p j n", j=NIN),
                    ee[:].rearrange("p (j n) -> p j n", j=NIN), rcpb, op=OP.mult)
                p2 = wp.tile([128, NIN * O], F32, tag="p2")
                for j in range(NIN):
                    rj = route[:, j * NC:(j + 1) * NC]
                    rjb = rj.unsqueeze(2).broadcast_to([128, NC, DC])
                    eng = nc.gpsimd if j < 4 else nc.vector
                    eng.tensor_tensor(
                        p2[:, j * O:(j + 1) * O].rearrange("p (n d) -> p n d", n=NC),
                        votes[:, j * O:(j + 1) * O].rearrange("p (n d) -> p n d", n=NC),
                        rjb, op=OP.mult)
                pre2 = smp.tile([128, O], F32, tag="pre2")
                nc.vector.reduce_sum(
                    pre2[:],
                    p2[:].rearrange("p (j n d) -> p n d j", j=NIN, n=NC),
                    axis=mybir.AxisListType.X)
                pre2b = smp.tile([128, O], F32, tag="pre2b")
                nc.vector.tensor_tensor(pre2b[:], pre2[:], bias[:], op=OP.add)

                # ---------------- final squash ----------------
                sqel2 = smp.tile([128, O], F32, tag="sqel2")
                nc.scalar.square(sqel2[:], pre2b[:])
                sq2 = smp.tile([128, NC], F32, tag="sq2")
                nc.vector.reduce_sum(
                    sq2[:], sqel2[:].rearrange("p (n d) -> p n d", d=DC),
                    axis=mybir.AxisListType.X)
                f2 = _squash_factor(nc, smp, sq2, "2")
                act2 = smp.tile([128, O], F32, tag="act2")
                f2b = f2[:].unsqueeze(2).broadcast_to([128, NC, DC])
                nc.vector.tensor_tensor(
                    act2[:].rearrange("p (n d) -> p n d", n=NC),
                    pre2b[:].rearrange("p (n d) -> p n d", n=NC), f2b, op=OP.mult)
                act2q = wp.tile([128, O], mybir.dt.uint8, tag="act2q")
                nc.vector.tensor_scalar(
                    act2q[:], act2[:], 127.0, 128.0, op0=OP.mult, op1=OP.add)

                nc.sync.dma_start(out.ap()[c * 128:(c + 1) * 128], act2q[:])

    nc.compile()
    return nc


def _squash_factor(nc, pool, sq, tag):
    """f = sq / ((1+sq) * sqrt(sq+EPS)), shape [128, NC]."""
    sqe = pool.tile([128, NC], F32, name=f"sqe{tag}", tag=f"sqe{tag}")
    nc.vector.tensor_scalar_add(sqe[:], sq[:], EPS)
    rt = pool.tile([128, NC], F32, name=f"rt{tag}", tag=f"rt{tag}")
    nc.scalar.activation(rt[:], sqe[:], AF.Sqrt)
    u = pool.tile([128, NC], F32, name=f"u{tag}", tag=f"u{tag}")
    nc.vector.tensor_scalar_add(u[:], sq[:], 1.0)
    w = pool.tile([128, NC], F32, name=f"w{tag}", tag=f"w{tag}")
    nc.vector.tensor_tensor(w[:], u[:], rt[:], op=OP.mult)
    vr = pool.tile([128, NC], F32, name=f"vr{tag}", tag=f"vr{tag}")
    nc.vector.reciprocal(vr[:], w[:])
    f = pool.tile([128, NC], F32, name=f"f{tag}", tag=f"f{tag}")
    nc.vector.tensor_tensor(f[:], sq[:], vr[:], op=OP.mult)
    return f


def make_inputs(x):
    """Per-core channel-major 12-bit packed layout with trailing dequant
    scale: xb[c][ch, pair*3 + k], tail = (s, -2048*s) as little-endian f32."""
    xt = np.ascontiguousarray(
        np.transpose(np.asarray(x, np.float32), (3, 4, 0, 1, 2))
    ).reshape(NCORES, DIN, NIN * IMG)
    amax = max(float(np.abs(xt).max()), 1e-30)
    s = amax / 2047.0
    q = np.clip(np.round(xt * (1.0 / s)) + 2048.0, 0.0, 4095.0).astype(np.uint16)
    q0, q1 = q[:, :, 0::2], q[:, :, 1::2]
    npair = NIN * IMG // 2
    xb = np.empty((NCORES, DIN, 3 * npair + 8), np.uint8)
    xb[:, :, 0:npair] = q0 & 255
    xb[:, :, npair:2 * npair] = q1 & 255
    xb[:, :, 2 * npair:3 * npair] = (q0 >> 8) + ((q1 >> 8) << 4)
    tail = np.frombuffer(
        np.array([s, -2048.0 * s], dtype="<f4").tobytes(), np.uint8)
    xb[:, :, 3 * npair:] = tail
    return [{"xb": xb[core]} for core in range(NCORES)]


def _get_module(W, b):
    key = (W.tobytes(), b.tobytes())
    ent = _CACHE.get("mod")
    if ent is None or ent[0] != key:
        nc = build_module(np.asarray(W, np.float32), np.asarray(b, np.float32))
        seq = _CACHE.get("modseq", 0) + 1
        _CACHE["modseq"] = seq
        try:
            nc._bass_memo_seq = seq
        except Exception:
            pass
        ent = (key, nc)
        _CACHE["mod"] = ent
    return ent[1]


def kernel(x, W, b):
    nc = _get_module(np.asarray(W, np.float32), np.asarray(b, np.float32))
    in_maps = make_inputs(x)
    res = bass_utils.run_bass_kernel_spmd(nc, in_maps, core_ids=list(range(NCORES)))
    outs = [res.results[c]["out"] for c in range(NCORES)]
    u8 = np.stack(outs, axis=0).reshape(B, H, W_, NC, DC)
    return (u8.astype(np.float32) - 128.0) * (1.0 / 127.0)
